# revision 66
# baseline (speedup 1.0000x reference)
"""Multi-head self-attention (B=2, S=2048, D=1024, H=16) on 8 trn2 cores.

Sharding: data-parallel over B (2) x tensor-parallel over head groups (4 groups
of 4 heads).  Core c handles batch c//4, heads (c%4)*4..(c%4)*4+3.  Each core
computes its partial output projection (over its 256 of the 1024 contraction
columns); the host sums the 4 partials per batch and adds the bias terms.

Structure (per core):
  - Inputs are host-transposed to partition-major [128, blk, cols] DRAM
    layouts so weights load in one large DMA each and x in 2KB-row halves
    (the two input DMA queues sustain ~220GB/s aggregate; descriptor count
    and row length both matter).
  - ~7us of zero-matmul warmup keeps the PE HAM clock gate at 8/8 through
    the DMA-paced startup; the prefix (K ec0+ec1 for keys 0-1023 and Q for
    query chunks 0-1, all phase-1 data) fills the PE while x streams in.
  - One merged 128-unit attention stream across all four query chunks (no
    per-chunk drain, so the ScalarE exp pipeline never goes idle at chunk
    boundaries).  Unit u: scores (2 row-group-packed bf16 K=64 matmuls into
    a [128,2heads,512] psum block), one 1024-wide exp on ScalarE, ctx
    matmuls issued DELAY units later (exp @ [ones|V] gives softmax
    denominators for free; V' garbage columns only pollute unread psum
    partitions).
  - Remaining K/V projections and the output projections are injected as
    fillers at stream slots chosen to meet their consumer deadlines.
  - Output partials leave as bf16 on both DMA queues (host reduces in
    fp64); warm-keeper matmuls bridge the final normalize chain so the last
    out-projections run at the warm clock.
"""

import sys

if "/opt/trn_rl_repo" not in sys.path:
    sys.path.insert(0, "/opt/trn_rl_repo")

from contextlib import ExitStack

import ml_dtypes
import numpy as np

import concourse.bass as bass
import concourse.mybir as mybir
import concourse.tile as tile
from concourse import bacc


F32 = mybir.dt.float32
F32R = mybir.dt.float32r
BF16 = mybir.dt.bfloat16
EXP = mybir.ActivationFunctionType.Exp

N_CORES = 8
S = 2048          # sequence length
D = 1024          # model dim
GH = 4            # heads per core
DK = 64           # head dim
E = GH * DK       # projection cols per core (256)
DT = D // 128     # contraction tiles over model dim (8)
QC = S // 512     # query chunks (4)
NU = 128          # total attention units (4 qc x 2 pairs x 16 kt)
DELAY = 8         # ctx trails scores by this many units.  8 balances ex
                  # lookahead (absorbs PE-vs-ScalarE jitter) against ctx
                  # compression at pair boundaries; 10 measured worse.


def _build():
    nc = bacc.Bacc("TRN2", target_bir_lowering=False, debug=False,
                   enable_asserts=False, num_devices=N_CORES)

    # All inputs are pre-laid-out on the host as [128 partitions, ...] with
    # long contiguous rows, so each tensor loads in O(1) large DMAs at full
    # HBM bandwidth (a DMA descriptor costs ~650ns regardless of size).
    xT_d = nc.dram_tensor("xT", [128, DT, S], BF16, kind="ExternalInput").ap()
    wqT_d = nc.dram_tensor("wqT", [128, DT, E], BF16, kind="ExternalInput").ap()
    wkT_d = nc.dram_tensor("wkT", [128, DT, E], BF16, kind="ExternalInput").ap()
    wvT_d = nc.dram_tensor("wvT", [128, DT, E], BF16, kind="ExternalInput").ap()
    woT_d = nc.dram_tensor("woT", [128, 2, D], BF16, kind="ExternalInput").ap()
    bq_d = nc.dram_tensor("bq", [E], F32, kind="ExternalInput").ap()
    bk_d = nc.dram_tensor("bk", [E], F32, kind="ExternalInput").ap()
    # partial sums leave as bf16: host adds 4 partials per batch in fp64, so
    # the rounding noise stays ~4e-3 rel l2 -- far inside the 2e-2 gate --
    # and the output DMA bytes (the serial tail of the kernel) halve.
    out_d = nc.dram_tensor("out", [S, D], BF16, kind="ExternalOutput").ap()

    with tile.TileContext(nc) as tc, ExitStack() as ctx:
        const = ctx.enter_context(tc.tile_pool(name="const", bufs=1))

        xT_s = const.tile([128, DT, S], BF16, name="xT_s")
        wqT_s = const.tile([128, DT, E], BF16, name="wqT_s")
        wkT_s = const.tile([128, DT, E], BF16, name="wkT_s")
        wvT_s = const.tile([128, DT, E], BF16, name="wvT_s")
        woT_s = const.tile([128, 2, D], BF16, name="woT_s")
        bq_s = const.tile([128, 2], F32, name="bq_s")
        bk_s = const.tile([128, 2], F32, name="bk_s")
        # Q and ctx live in per-qc tiles to avoid cross-qc WAR serialization
        QT_q = [const.tile([128, 2, 512], BF16, name=f"QT{i}") for i in range(QC)]
        ctx_q = [[const.tile([128, 512], BF16, name=f"CT{i}_{p}") for p in range(2)]
                 for i in range(QC)]
        KT_p = [const.tile([128, 1024], BF16, name=f"KTp{i}") for i in range(4)]
        # V' layout: [k-partition, k-chunk, head-major 128-col blocks].  Per
        # head block: col 0 = ones (softmax denominator), cols 64..127 = V
        # (DVE 64-partition reads must start at 0/64); cols 1..63 are never
        # zeroed -- garbage there only pollutes psum partitions we never read.
        Vp_t = [const.tile([128, 4, GH * 128], BF16, name=f"Vp{i}")
                for i in range(4)]
        junk = const.tile([128, 512], BF16, name="junk")

        # ---- DMA: two queues, strict phase order ---------------------------
        # phase 1 (prefix-critical): wk | wq whole-tensor, then per-dt x
        # first halves (cols 0-1023, 2KB rows) alternating queues.
        # phase 2: wv first (vproj fillers start at unit 1), x second halves,
        # wo.
        nc.sync.dma_start(bk_s[:, :], bk_d.rearrange("(c p) -> p c", p=128))
        nc.gpsimd.dma_start(bq_s[:, :], bq_d.rearrange("(c p) -> p c", p=128))
        nc.sync.dma_start(wkT_s[:, :, :], wkT_d[:, :, :])
        nc.gpsimd.dma_start(wqT_s[:, :, :], wqT_d[:, :, :])
        for dt_ in range(DT):
            eng = nc.sync if dt_ % 2 == 0 else nc.gpsimd
            eng.dma_start(xT_s[:, dt_, 0:1024], xT_d[:, dt_, 0:1024])
        nc.sync.dma_start(wvT_s[:, :, :], wvT_d[:, :, :])
        for dt_ in range(DT):
            eng = nc.gpsimd if dt_ % 2 == 0 else nc.sync
            eng.dma_start(xT_s[:, dt_, 1024:2048], xT_d[:, dt_, 1024:2048])
        nc.gpsimd.dma_start(woT_s[:, :, :], woT_d[:, :, :])

        # ones columns for the softmax denominators (DVE, no DMA deps)
        nc.vector.memset(junk[:], 0.0)
        for vt in Vp_t:
            for hh in range(GH):
                nc.vector.memset(vt[:, :, hh * 128], 1.0)

        sp = ctx.enter_context(tc.tile_pool(name="sp", bufs=3, space="PSUM"))
        cp = ctx.enter_context(tc.tile_pool(name="cp", bufs=1, space="PSUM"))
        ep = ctx.enter_context(tc.tile_pool(name="ep", bufs=12))
        npool = ctx.enter_context(tc.tile_pool(name="npool", bufs=3))
        op = ctx.enter_context(tc.tile_pool(name="op", bufs=4))

        # ---- PE warmup: ~3.4us of zero matmuls with no DMA deps, so the HAM
        # clock gate reaches 8/8 before the real DMA-paced prefix runs.  N=256
        # keeps the warmup short enough not to delay the prefix in the
        # in-order PE queue.
        wt = sp.tile([128, 2, 512], F32, name="sblk")
        for _ in range(32):
            nc.tensor.matmul(wt[:, 0, 0:256], junk[:, 0:128], junk[:, 0:256],
                             start=True, stop=True)

        # ---- startup prefix --------------------------------------------------
        # Everything that needs only phase-1 DMA data (wk, wq, x cols 0-1023):
        # K ec0+ec1 for keys 0-1023 and the Q projections for query chunks 0
        # and 1.  The PE would idle ~40% of the DMA-paced window otherwise, so
        # these 64 matmuls are nearly free wall-clock and they halve the
        # filler load of the first query chunk.  The 4th psum tile borrows
        # the (still idle) ctx accumulator pool.
        ps_k = sp.tile([128, 2, 512], F32, name="sblk")
        ps_q = sp.tile([128, 2, 512], F32, name="sblk")
        ps_k2 = sp.tile([128, 2, 512], F32, name="sblk")
        ps_q2 = cp.tile([128, 2, 512], F32, name="cps")
        for dt_ in range(DT):
            st, fin = dt_ == 0, dt_ == DT - 1
            nc.tensor.matmul(ps_k[:, 0, :], wkT_s[:, dt_, 0:128],
                             xT_s[:, dt_, 0:512], start=st, stop=fin)
            nc.tensor.matmul(ps_k[:, 1, :], wkT_s[:, dt_, 0:128],
                             xT_s[:, dt_, 512:1024], start=st, stop=fin)
            nc.tensor.matmul(ps_q[:, 0, :], wqT_s[:, dt_, 0:128],
                             xT_s[:, dt_, 0:512], start=st, stop=fin)
            nc.tensor.matmul(ps_q[:, 1, :], wqT_s[:, dt_, 128:256],
                             xT_s[:, dt_, 0:512], start=st, stop=fin)
            nc.tensor.matmul(ps_k2[:, 0, :], wkT_s[:, dt_, 128:256],
                             xT_s[:, dt_, 0:512], start=st, stop=fin)
            nc.tensor.matmul(ps_k2[:, 1, :], wkT_s[:, dt_, 128:256],
                             xT_s[:, dt_, 512:1024], start=st, stop=fin)
            nc.tensor.matmul(ps_q2[:, 0, :], wqT_s[:, dt_, 0:128],
                             xT_s[:, dt_, 512:1024], start=st, stop=fin)
            nc.tensor.matmul(ps_q2[:, 1, :], wqT_s[:, dt_, 128:256],
                             xT_s[:, dt_, 512:1024], start=st, stop=fin)
        # eviction order matters: scores(0) needs only QT_q[0] ec0 + K keys
        # 0-511 of ec0
        KT0v = KT_p[0].rearrange("p (a b) -> p a b", a=2)
        KT2v = KT_p[2].rearrange("p (a b) -> p a b", a=2)
        nc.vector.tensor_scalar_add(QT_q[0][:, 0, :], ps_q[:, 0, :], bq_s[:, 0:1])
        nc.vector.tensor_scalar_add(KT0v[:, 0, :], ps_k[:, 0, :], bk_s[:, 0:1])
        nc.vector.tensor_scalar_add(KT0v[:, 1, :], ps_k[:, 1, :], bk_s[:, 0:1])
        nc.vector.tensor_scalar_add(QT_q[0][:, 1, :], ps_q[:, 1, :], bq_s[:, 1:2])
        nc.vector.tensor_scalar_add(KT2v[:], ps_k2[:], bk_s[:, 1:2])
        nc.vector.tensor_scalar_add(QT_q[1][:, 0, :], ps_q2[:, 0, :], bq_s[:, 0:1])
        nc.vector.tensor_scalar_add(QT_q[1][:, 1, :], ps_q2[:, 1, :], bq_s[:, 1:2])

        # ---- filler building blocks (all use the sp psum ring) -------------
        def kproj_kq(ec, kq):
            ps = sp.tile([128, 2, 512], F32, name="sblk")
            for half in range(2):
                qcol = kq * 2 + half
                for dt_ in range(DT):
                    nc.tensor.matmul(
                        ps[:, half, :],
                        wkT_s[:, dt_, ec * 128:(ec + 1) * 128],
                        xT_s[:, dt_, qcol * 512:(qcol + 1) * 512],
                        start=(dt_ == 0), stop=(dt_ == DT - 1))
            nc.vector.tensor_scalar_add(
                KT_p[ec * 2 + kq].rearrange("p (a b) -> p a b", a=2),
                ps[:], bk_s[:, ec:ec + 1])

        def vproj_sg(sg):
            ps = sp.tile([128, 2, 512], F32, name="sblk")
            for half in range(2):
                sc = sg * 2 + half
                for dt_ in range(DT):
                    nc.tensor.matmul(
                        ps[:, half, :E],
                        xT_s[:, dt_, sc * 128:(sc + 1) * 128],
                        wvT_s[:, dt_, :],
                        start=(dt_ == 0), stop=(dt_ == DT - 1))
            vt, so = Vp_t[sg // 2], (sg % 2) * 2
            nc.vector.tensor_copy(
                vt[:, so:so + 2, :]
                .rearrange("p s (h d) -> p s h d", d=128)[:, :, :, DK:128],
                ps[:, :, :E].rearrange("p s (h d) -> p s h d", d=DK))

        def qproj_ec(qc, ec):
            ps = sp.tile([128, 2, 512], F32, name="sblk")
            for dt_ in range(DT):
                nc.tensor.matmul(
                    ps[:, ec, :],
                    wqT_s[:, dt_, ec * 128:(ec + 1) * 128],
                    xT_s[:, dt_, qc * 512:(qc + 1) * 512],
                    start=(dt_ == 0), stop=(dt_ == DT - 1))
            nc.vector.tensor_scalar_add(
                QT_q[qc][:, ec, :], ps[:, ec, :], bq_s[:, ec:ec + 1])

        def outproj_si(qc, si, scalar_cast=False):
            ssl = slice(si * 128, (si + 1) * 128)
            os_ = op.tile([128, D], BF16, name="os_")
            ps = sp.tile([128, 2, 512], F32, name="sblk")
            for eh in range(2):
                for dc in range(2):
                    nc.tensor.matmul(
                        ps[:, eh, :],
                        ctx_q[qc][dc][:, ssl],
                        woT_s[:, dc, eh * 512:(eh + 1) * 512],
                        start=(dc == 0), stop=(dc == 1))
            # tail outprojs alternate their evictions between ScalarE (idle
            # after its last exp) and the DVE (free once the finish-chain
            # muls complete) so the four casts run two-wide
            if scalar_cast and si % 2 == 0:
                nc.scalar.copy(os_[:], ps[:].rearrange("p a b -> p (a b)"))
            else:
                nc.vector.tensor_copy(os_[:], ps[:].rearrange("p a b -> p (a b)"))
            row = qc * 512 + si * 128
            if scalar_cast:
                # tail: the scalar queue is idle too, spread the final drain
                # 3-wide so the closing barrier fires sooner
                eng = (nc.sync, nc.gpsimd, nc.scalar, nc.gpsimd)[si]
            else:
                eng = nc.sync if si % 2 == 0 else nc.gpsimd
            eng.dma_start(out_d[row:row + 64, :], os_[0:64, :])
            eng.dma_start(out_d[row + 64:row + 128, :], os_[64:128, :])

        # ---- merged attention stream ---------------------------------------
        # Filler slots chosen so every producer lands ~2.5 units before its
        # first consumer (completion model: MMs ~+1.5u, DVE eviction ~+1u).
        def upair(u):
            return u // 32, (u % 32) // 16, u % 16

        fillers = {
            1: lambda: vproj_sg(0), 3: lambda: vproj_sg(1),
            5: lambda: kproj_kq(0, 1), 7: lambda: vproj_sg(2),
            9: lambda: vproj_sg(3), 11: lambda: vproj_sg(4),
            13: lambda: kproj_kq(1, 1), 15: lambda: vproj_sg(5),
            17: lambda: vproj_sg(6), 19: lambda: vproj_sg(7),
            45: lambda: outproj_si(0, 0), 49: lambda: qproj_ec(2, 0),
            53: lambda: qproj_ec(2, 1), 57: lambda: outproj_si(0, 1),
            61: lambda: outproj_si(0, 2), 65: lambda: outproj_si(0, 3),
            77: lambda: outproj_si(1, 0), 81: lambda: qproj_ec(3, 0),
            85: lambda: qproj_ec(3, 1), 89: lambda: outproj_si(1, 1),
            93: lambda: outproj_si(1, 2), 97: lambda: outproj_si(1, 3),
            107: lambda: outproj_si(2, 0), 111: lambda: outproj_si(2, 1),
            115: lambda: outproj_si(2, 2), 119: lambda: outproj_si(2, 3),
        }

        cps = {}
        exs = {}

        def do_scores(u):
            qc, pair, kt = upair(u)
            sblk = sp.tile([128, 2, 512], F32, name="sblk")
            for hi in range(2):
                po = hi * 64
                nc.tensor.matmul(
                    sblk[:, hi, :],
                    KT_p[pair * 2 + kt // 8][po:po + 64,
                                             (kt % 8) * 128:(kt % 8 + 1) * 128],
                    QT_q[qc][po:po + 64, pair, :],
                    start=True, stop=True)
            ex = ep.tile([128, 2, 512], BF16, name="ex")
            nc.scalar.activation(ex[:], sblk[:], EXP, scale=0.125)
            exs[u] = ex

        def finish_pair(qc, pair):
            last = (qc == QC - 1 and pair == 1)
            cc = cps.pop((qc, pair))
            if last:
                cu = cc  # read psum directly; no next pair needs the slot
            else:
                cu = npool.tile([128, 2, 512], F32, name="cu")
                nc.vector.tensor_copy(cu[:], cc[:, :, :])
            rc = npool.tile([1, 1024], F32, name="rc")
            bc = npool.tile([128, 1024], F32, name="bc")
            if last:
                # split per head so broadcast(hi0) overlaps reciprocal(hi1);
                # this chain is the serial tail of the whole kernel
                for hi in range(2):
                    nc.vector.reciprocal_approx_fast(
                        rc[:, hi * 512:(hi + 1) * 512], cu[0:1, hi, :])
                    nc.gpsimd.partition_broadcast(
                        bc[:, hi * 512:(hi + 1) * 512],
                        rc[:, hi * 512:(hi + 1) * 512])
            else:
                nc.vector.reciprocal_approx_fast(
                    rc[:], cu[0:1, :, :].rearrange("p a b -> p (a b)"))
                for hi in range(2):
                    nc.gpsimd.partition_broadcast(
                        bc[:, hi * 512:(hi + 1) * 512],
                        rc[:, hi * 512:(hi + 1) * 512])
            for hi in range(2):
                po = hi * 64
                nc.vector.tensor_mul(
                    ctx_q[qc][pair][po:po + 64, :], cu[64:128, hi, :],
                    bc[64:128, hi * 512:(hi + 1) * 512])

        def do_ctx(u):
            qc, pair, kt = upair(u)
            ex = exs.pop(u)
            if kt == 0:
                cps[(qc, pair)] = cp.tile([128, 2, 512], F32, name="cps")
            cc = cps[(qc, pair)]
            for hi in range(2):
                h = pair * 2 + hi
                nc.tensor.matmul(
                    cc[:, hi, :],
                    Vp_t[kt // 4][:, kt % 4, h * 128:(h + 1) * 128],
                    ex[:, hi, :],
                    start=(kt == 0), stop=(kt == 15))
            if kt == 15:
                finish_pair(qc, pair)

        for u in range(NU + DELAY):
            if u < NU:
                do_scores(u)
            if u >= DELAY:
                do_ctx(u - DELAY)
            f = fillers.pop(u, None)
            if f is not None:
                f()

        # Tail: the final out projections' pair-0 halves depend only on
        # ctx_q[3][0] (ready long before the last exp), so they run during
        # the pair-1 normalize chain -- real work replacing the warm-keeper,
        # and the post-normalize critical path halves.  Three slices fit the
        # psum ring; slice 3 runs as a normal unit afterwards.
        tail_ps = []
        for si in range(3):
            ps = sp.tile([128, 2, 512], F32, name="sblk")
            ssl = slice(si * 128, (si + 1) * 128)
            for eh in range(2):
                nc.tensor.matmul(
                    ps[:, eh, :], ctx_q[QC - 1][0][:, ssl],
                    woT_s[:, 0, eh * 512:(eh + 1) * 512],
                    start=True, stop=False)
            tail_ps.append(ps)
        for si in range(3):
            ps = tail_ps[si]
            ssl = slice(si * 128, (si + 1) * 128)
            os_ = op.tile([128, D], BF16, name="os_")
            for eh in range(2):
                nc.tensor.matmul(
                    ps[:, eh, :], ctx_q[QC - 1][1][:, ssl],
                    woT_s[:, 1, eh * 512:(eh + 1) * 512],
                    start=False, stop=True)
            if si % 2 == 0:
                nc.scalar.copy(os_[:], ps[:].rearrange("p a b -> p (a b)"))
            else:
                nc.vector.tensor_copy(os_[:], ps[:].rearrange("p a b -> p (a b)"))
            row = (QC - 1) * 512 + si * 128
            eng = (nc.sync, nc.gpsimd, nc.scalar)[si]
            eng.dma_start(out_d[row:row + 64, :], os_[0:64, :])
            eng.dma_start(out_d[row + 64:row + 128, :], os_[64:128, :])
        outproj_si(QC - 1, 3, scalar_cast=True)

    nc.compile()
    return nc


_STATE = {}


def _get_nc():
    if "nc" not in _STATE:
        _STATE["nc"] = _build()
    return _STATE["nc"]


def kernel(x, wq, bq, wk, bk, wv, bv, wo, bo):
    x = np.asarray(x, dtype=np.float32)
    wq = np.asarray(wq, dtype=np.float32)
    bq = np.asarray(bq, dtype=np.float32)
    wk = np.asarray(wk, dtype=np.float32)
    bk = np.asarray(bk, dtype=np.float32)
    wv = np.asarray(wv, dtype=np.float32)
    bv = np.asarray(bv, dtype=np.float32)
    wo = np.asarray(wo, dtype=np.float32)
    bo = np.asarray(bo, dtype=np.float32)

    nc = _get_nc()

    def pmaj(a, nblk):
        # [nblk*128, cols] -> [128, nblk, cols] partition-major layout
        return np.ascontiguousarray(
            a.reshape(nblk, 128, a.shape[1]).transpose(1, 0, 2)
        ).astype(ml_dtypes.bfloat16)

    in_maps = []
    for c in range(N_CORES):
        b, g = divmod(c, 4)
        cols = slice(g * E, (g + 1) * E)
        in_maps.append({
            "xT": pmaj(x[b].T, DT),
            "wqT": pmaj(wq[cols, :].T, DT),
            "wkT": pmaj(wk[cols, :].T, DT),
            "wvT": pmaj(wv[cols, :].T, DT),
            "woT": pmaj(wo[:, cols].T, 2),
            "bq": np.ascontiguousarray(bq[cols]),
            "bk": np.ascontiguousarray(bk[cols]),
        })

    from concourse import bass_utils
    res = bass_utils.run_bass_kernel_spmd(
        nc, in_maps, core_ids=list(range(N_CORES)), trace=False)

    bias = (bo + wo @ bv).astype(np.float32)
    out = np.empty((2, S, D), dtype=np.float32)
    for b in range(2):
        acc = res.results[b * 4 + 0]["out"].astype(np.float64)
        for g in range(1, 4):
            acc += res.results[b * 4 + g]["out"]
        out[b] = (acc + bias).astype(np.float32)
    return out
